# revision 1
# baseline (speedup 1.0000x reference)
"""Two-layer GraphSAGE (mean aggregation) on 8 Trainium2 NeuronCores.

Strategy (matches the dst-partitioning hint):
- Nodes are partitioned by destination across 8 cores (12500 nodes each,
  padded to 12544 = 98*128). Each core owns the edges whose dst lands in
  its slice, pre-sorted/bucketed by (core, dst-tile) on the host.
- x is replicated to every core in a padded layout so src indices are
  identical for both layers. Layer-1 aggregation gathers x[src] rows with
  large indirect DMAs, reduces them per 128-dst tile with indicator
  matmuls on the TensorEngine (indicator built on DVE from host-provided
  dst offsets), and applies mean + the two dense 128x128 matmuls.
- Between layers, each core's h slice is AllGathered so layer 2 can
  gather h[src] for remote sources. Layer-2 self term reads the local
  (pre-AllGather) slice.

kernel(**inputs) -> np.ndarray takes the FULL inputs and returns the FULL
[100000, 128] output; all sharding/unsharding happens inside.
"""

import math
import os

import numpy as np

P = 128
NCORES = 8


def _prep_edges(edge_index: np.ndarray, n_nodes: int, npc: int, tpc: int):
    """Bucket edges by (owner core, dst tile); pad each tile to whole
    128-edge chunks (uniform chunk count across cores per tile so the SPMD
    program is identical on every core).

    Returns (ch, coloff, ncols, esrc, edst):
      ch[t]    : number of 128-edge chunks for dst tile t (max over cores)
      coloff[t]: starting column of tile t in the packed arrays
      esrc     : [8, 128, ncols] int32, padded-global src ids (pad = 0)
      edst     : [8, 128, ncols] float32, dst offset within tile (pad = -1)
    """
    npc_pad = tpc * P
    src = edge_index[0].astype(np.int64)
    dst = edge_index[1].astype(np.int64)
    srcpad = ((src // npc) * npc_pad + (src % npc)).astype(np.int64)
    core = dst // npc
    loc = dst % npc
    tl = loc // P
    off = loc % P

    key = core * tpc + tl
    counts = np.bincount(key, minlength=NCORES * tpc).reshape(NCORES, tpc)
    ch = np.maximum(1, -(-counts.max(axis=0) // P)).astype(np.int64)
    coloff = np.zeros(tpc + 1, np.int64)
    np.cumsum(ch, out=coloff[1:])
    ncols = int(coloff[-1])

    esrc = np.zeros((NCORES, ncols * P), np.int32)
    edst = np.full((NCORES, ncols * P), -1.0, np.float32)

    order = np.argsort(key, kind="stable")
    sk = key[order]
    first = np.r_[True, sk[1:] != sk[:-1]]
    idx_of_first = np.where(first)[0]
    grp_id = np.cumsum(first) - 1
    rank = np.arange(len(sk)) - idx_of_first[grp_id]
    slot = coloff[tl[order]] * P + rank
    esrc[core[order], slot] = srcpad[order].astype(np.int32)
    edst[core[order], slot] = off[order].astype(np.float32)

    esrc = np.ascontiguousarray(esrc.reshape(NCORES, ncols, P).transpose(0, 2, 1))
    edst = np.ascontiguousarray(edst.reshape(NCORES, ncols, P).transpose(0, 2, 1))

    # per-node 1/max(indegree,1), laid out [core][partition, tile]
    cnt = np.bincount(dst, minlength=n_nodes).astype(np.float32)
    recip = np.zeros((NCORES, npc_pad), np.float32)
    for c in range(NCORES):
        recip[c, :npc] = 1.0 / np.maximum(cnt[c * npc : (c + 1) * npc], 1.0)
    recip = np.ascontiguousarray(recip.reshape(NCORES, tpc, P).transpose(0, 2, 1))
    return ch, coloff, ncols, esrc, edst, recip


def _gather_groups(ch, coloff, tpc, gmax):
    """Group consecutive dst tiles so each group's gather is one indirect
    DMA of at most gmax columns (gmax*128 rows)."""
    groups = []
    t = 0
    while t < tpc:
        t0 = t
        cols = 0
        while t < tpc and cols + ch[t] <= gmax:
            cols += ch[t]
            t += 1
        groups.append((t0, t, int(coloff[t0]), int(coloff[t])))
    return groups


def _build_program(tpc, ncols, ch, coloff, groups, n_all_pad):
    from concourse import bacc, bass, mybir, tile

    npc_pad = tpc * P
    f32 = mybir.dt.float32
    i32 = mybir.dt.int32

    nc = bacc.Bacc(
        "TRN2", target_bir_lowering=False, debug=False, num_devices=NCORES
    )

    xg = nc.declare_dram_parameter("xg", [n_all_pad, P], f32, isOutput=False)
    xown = nc.declare_dram_parameter("xown", [npc_pad, P], f32, isOutput=False)
    esrc_d = nc.declare_dram_parameter("esrc", [P, ncols], i32, isOutput=False)
    edst_d = nc.declare_dram_parameter("edst", [P, ncols], f32, isOutput=False)
    wl1_d = nc.declare_dram_parameter("wl1", [P, P], f32, isOutput=False)
    wr1_d = nc.declare_dram_parameter("wr1", [P, P], f32, isOutput=False)
    wl2_d = nc.declare_dram_parameter("wl2", [P, P], f32, isOutput=False)
    wr2_d = nc.declare_dram_parameter("wr2", [P, P], f32, isOutput=False)
    bias1_d = nc.declare_dram_parameter("bias1", [P, P], f32, isOutput=False)
    bias2_d = nc.declare_dram_parameter("bias2", [P, P], f32, isOutput=False)
    iota_d = nc.declare_dram_parameter("iota", [P, P], f32, isOutput=False)
    ident_d = nc.declare_dram_parameter("ident", [P, P], f32, isOutput=False)
    recip_d = nc.declare_dram_parameter("recip", [P, tpc], f32, isOutput=False)
    out_d = nc.declare_dram_parameter("out", [npc_pad, P], f32, isOutput=True)

    gmax = max(g[3] - g[2] for g in groups)

    with tile.TileContext(nc) as tc:
        with (
            tc.tile_pool(name="const", bufs=1) as cpool,
            tc.tile_pool(name="meta", bufs=1) as mpool,
            tc.tile_pool(name="gath", bufs=2) as gpool,
            tc.tile_pool(name="work", bufs=3) as wpool,
            tc.tile_pool(name="psacc", bufs=2, space="PSUM") as ps_acc,
            tc.tile_pool(name="psself", bufs=2, space="PSUM") as ps_self,
            tc.tile_pool(name="psh", bufs=2, space="PSUM") as ps_h,
            tc.tile_pool(name="dram", bufs=1, space="DRAM") as dpool,
        ):
            def load_const(dram_ap, shape, dtype=f32, name=None):
                t = cpool.tile(shape, dtype, name=name)
                nc.sync.dma_start(out=t[:], in_=dram_ap)
                return t

            wl1 = load_const(wl1_d[:], [P, P], name="wl1")
            wr1 = load_const(wr1_d[:], [P, P], name="wr1")
            wl2 = load_const(wl2_d[:], [P, P], name="wl2")
            wr2 = load_const(wr2_d[:], [P, P], name="wr2")
            bias1 = load_const(bias1_d[:], [P, P], name="bias1")
            bias2 = load_const(bias2_d[:], [P, P], name="bias2")
            iota = load_const(iota_d[:], [P, P], name="iota")
            ident = load_const(ident_d[:], [P, P], name="ident")
            recip = load_const(recip_d[:], [P, tpc], name="recip")
            esrc = mpool.tile([P, ncols], i32, name="esrc")
            nc.sync.dma_start(out=esrc[:], in_=esrc_d[:])
            edst = mpool.tile([P, ncols], f32, name="edst")
            nc.sync.dma_start(out=edst[:], in_=edst_d[:])

            h_bounce = dpool.tile([npc_pad, P], f32, name="h_bounce")
            h_full = dpool.tile(
                [n_all_pad, P], f32, name="h_full", addr_space="Shared"
            )

            def layer(src_table, self_src, dst_dram, wl, wr, bias, relu):
                for (t0, t1, c0, c1) in groups:
                    g_sb = gpool.tile([P, gmax * P], f32, tag="gath")
                    # HW indirect DMA consumes ONE offset per partition, so
                    # gather 128 rows per instruction (one per chunk column).
                    for cc in range(c0, c1):
                        nc.gpsimd.indirect_dma_start(
                            out=g_sb[:, (cc - c0) * P : (cc - c0 + 1) * P],
                            out_offset=None,
                            in_=src_table[:],
                            in_offset=bass.IndirectOffsetOnAxis(
                                ap=esrc[:, cc : cc + 1], axis=0
                            ),
                        )
                    for t in range(t0, t1):
                        cht = int(ch[t])
                        tc0 = int(coloff[t]) - c0
                        ind = wpool.tile([P, cht, P], f32, tag="ind")
                        nc.vector.tensor_tensor(
                            out=ind[:],
                            in0=edst[:, coloff[t] : coloff[t] + cht, None]
                            .to_broadcast([P, cht, P]),
                            in1=iota[:, None, :].to_broadcast([P, cht, P]),
                            op=mybir.AluOpType.is_equal,
                        )
                        acc = ps_acc.tile([P, P], f32, tag="acc")
                        for k in range(cht):
                            nc.tensor.matmul(
                                out=acc[:],
                                lhsT=g_sb[:, (tc0 + k) * P : (tc0 + k + 1) * P],
                                rhs=ind[:, k, :],
                                start=(k == 0),
                                stop=(k == cht - 1),
                            )
                        # self term: x_own[t] transposed via PE
                        xo = wpool.tile([P, P], f32, tag="xo")
                        nc.sync.dma_start(
                            out=xo[:], in_=self_src[t * P : (t + 1) * P, :]
                        )
                        selfT_ps = ps_self.tile([P, P], f32, tag="selfT")
                        nc.tensor.transpose(
                            out=selfT_ps[:], in_=xo[:], identity=ident[:]
                        )
                        selfT = wpool.tile([P, P], f32, tag="selfT_sb")
                        nc.vector.tensor_copy(out=selfT[:], in_=selfT_ps[:])
                        aggT = wpool.tile([P, P], f32, tag="aggT_sb")
                        nc.vector.tensor_copy(out=aggT[:], in_=acc[:])
                        h1 = ps_h.tile([P, P], f32, tag="h1")
                        nc.tensor.matmul(
                            out=h1[:], lhsT=aggT[:], rhs=wl[:],
                            start=True, stop=True,
                        )
                        h2 = ps_h.tile([P, P], f32, tag="h2")
                        nc.tensor.matmul(
                            out=h2[:], lhsT=selfT[:], rhs=wr[:],
                            start=True, stop=True,
                        )
                        hsb = wpool.tile([P, P], f32, tag="hsb")
                        nc.vector.tensor_scalar_mul(
                            out=hsb[:], in0=h1[:], scalar1=recip[:, t : t + 1]
                        )
                        nc.vector.tensor_add(out=hsb[:], in0=hsb[:], in1=h2[:])
                        nc.vector.tensor_add(out=hsb[:], in0=hsb[:], in1=bias[:])
                        if relu:
                            nc.scalar.activation(
                                out=hsb[:], in_=hsb[:],
                                func=mybir.ActivationFunctionType.Relu,
                            )
                        nc.sync.dma_start(
                            out=dst_dram[t * P : (t + 1) * P, :], in_=hsb[:]
                        )

            layer(xg, xown, h_bounce, wl1, wr1, bias1, relu=True)
            nc.gpsimd.collective_compute(
                "AllGather",
                mybir.AluOpType.bypass,
                replica_groups=[list(range(NCORES))],
                ins=[h_bounce[:]],
                outs=[h_full[:]],
            )
            layer(h_full, h_bounce, out_d, wl2, wr2, bias2, relu=False)

    return nc


def run(x, edge_index, W_l1, b_l1, W_r1, W_l2, b_l2, W_r2, trace=False):
    n_nodes = x.shape[0]
    assert n_nodes % NCORES == 0
    npc = n_nodes // NCORES
    tpc = -(-npc // P)
    npc_pad = tpc * P
    n_all_pad = NCORES * npc_pad
    gmax = int(os.environ.get("SAGE_GMAX", "24"))

    ch, coloff, ncols, esrc, edst, recip = _prep_edges(
        edge_index, n_nodes, npc, tpc
    )
    groups = _gather_groups(ch, coloff, tpc, gmax)

    x = np.asarray(x, np.float32)
    x_pad = np.zeros((n_all_pad, P), np.float32)
    for c in range(NCORES):
        x_pad[c * npc_pad : c * npc_pad + npc] = x[c * npc : (c + 1) * npc]

    common = {
        "xg": x_pad,
        "wl1": np.asarray(W_l1, np.float32),
        "wr1": np.asarray(W_r1, np.float32),
        "wl2": np.asarray(W_l2, np.float32),
        "wr2": np.asarray(W_r2, np.float32),
        "bias1": np.ascontiguousarray(
            np.broadcast_to(np.asarray(b_l1, np.float32), (P, P))
        ),
        "bias2": np.ascontiguousarray(
            np.broadcast_to(np.asarray(b_l2, np.float32), (P, P))
        ),
        "iota": np.ascontiguousarray(
            np.broadcast_to(np.arange(P, dtype=np.float32), (P, P))
        ),
        "ident": np.eye(P, dtype=np.float32),
    }
    in_maps = []
    for c in range(NCORES):
        m = dict(common)
        m["xown"] = np.ascontiguousarray(x_pad[c * npc_pad : (c + 1) * npc_pad])
        m["esrc"] = esrc[c]
        m["edst"] = edst[c]
        m["recip"] = recip[c]
        in_maps.append(m)

    nc = _build_program(tpc, ncols, ch, coloff, groups, n_all_pad)
    nc.finalize()

    from concourse.bass_utils import run_bass_kernel_spmd

    res = run_bass_kernel_spmd(
        nc, in_maps, list(range(NCORES)), trace=trace,
    )
    out = np.empty((n_nodes, P), np.float32)
    for c in range(NCORES):
        out[c * npc : (c + 1) * npc] = res.results[c]["out"][:npc]
    return out, res


def kernel(x, edge_index, W_l1, b_l1, W_r1, W_l2, b_l2, W_r2):
    out, _ = run(x, edge_index, W_l1, b_l1, W_r1, W_l2, b_l2, W_r2)
    return out



# revision 4
# speedup vs baseline: 1.1025x; 1.1025x over previous
"""Two-layer GraphSAGE (mean aggregation) on 8 Trainium2 NeuronCores.

Strategy (dst-partitioned, graph/data parallel):
- Nodes partitioned by destination across 8 cores (12500 each, padded to
  12544 = 98*128). x replicated per core in fp16 with a padded row space so
  src ids are layer-invariant.
- Edges bucketed per core by (group of 14 dst tiles, src quarter) and sorted
  by dst tile within each bucket. Source rows are fetched with a few large
  SWDGE dma_gather calls (int16 indices -> the padded node table is split
  into 4 quarters of 25088 rows each). Slot layout is identical on every
  core (capacity = max count over cores, pad slots gather row 0 with
  indicator weight 0) so the SPMD program is uniform.
- Aggregation: per (column, dst tile) incidence, one DVE tensor_scalar
  builds the indicator (iota == edst) * recip_indegree in one op, and one
  PE matmul accumulates [feat, dst] into PSUM. Mean normalization rides in
  the indicator values.
- h tile = bias (1-row matmul) + aggT @ W_l + x_selfT @ W_r accumulated in
  one PSUM tile; ReLU + cast on the Scalar engine. Self term loaded
  pre-transposed via HWDGE dma_start_transpose (fp16).
- AllGather (fp16) exchanges h between layers.

kernel(**inputs) -> np.ndarray takes FULL inputs, returns FULL [100000, 128]
float32 output; all sharding happens inside.
"""

import numpy as np

P = 128
NCORES = 8
NPC = 12500
TPC = 98
NPC_PAD = TPC * P            # 12544
NALL = NCORES * NPC_PAD      # 100352
NQ = 4
QROWS = NALL // NQ           # 25088
TG = 14                      # dst tiles per group
NG = TPC // TG               # 7 groups
GB = 2                       # groups per gather block
NB = -(-NG // GB)            # 4 gather blocks


def _prep(edge_index):
    """Host-side edge bucketing. Returns per-core SBUF tables and the
    core-independent program structure."""
    src = edge_index[0].astype(np.int64)
    dst = edge_index[1].astype(np.int64)
    core = dst // NPC
    loc = dst % NPC
    tl = loc // P
    off = loc % P
    g = tl // TG
    tl_loc = tl % TG
    srcpad = (src // NPC) * NPC_PAD + (src % NPC)
    q = srcpad // QROWS
    qrow = srcpad % QROWS

    # per (core, g, q, tile) counts -> capacities (max over cores)
    key = ((core * NG + g) * NQ + q) * TG + tl_loc
    cnt = np.bincount(key, minlength=NCORES * NG * NQ * TG).reshape(
        NCORES, NG, NQ, TG
    )
    cap = cnt.max(axis=0)                       # [NG, NQ, TG]
    scum = np.zeros((NG, NQ, TG + 1), np.int64)
    np.cumsum(cap, axis=2, out=scum[:, :, 1:])
    segslots = scum[:, :, TG]                   # [NG, NQ]
    segcols = -(-segslots // P)                 # [NG, NQ]

    # call layout: for block b, for q, for g in block -> contiguous columns
    blocks = [list(range(b * GB, min((b + 1) * GB, NG))) for b in range(NB)]
    colbase = np.zeros((NG, NQ), np.int64)      # global col of (g, q) seg
    calls = []                                  # (b, q, gstart_col, callcols)
    ncols = 0
    for b, gs in enumerate(blocks):
        for qq in range(NQ):
            c0 = ncols
            for gg in gs:
                colbase[gg, qq] = ncols
                ncols += int(segcols[gg, qq])
            calls.append((b, qq, c0, ncols - c0))

    # per-edge slot assignment (identical layout on every core)
    order = np.lexsort((tl_loc, q, g, core))
    sk = key[order]
    first = np.r_[True, sk[1:] != sk[:-1]]
    idx_of_first = np.where(first)[0]
    grp_id = np.cumsum(first) - 1
    rank = np.arange(len(sk)) - idx_of_first[grp_id]
    go, qo, to, co = g[order], q[order], tl_loc[order], core[order]
    slot = scum[go, qo, to] + rank
    gcol = colbase[go, qo] + slot // P
    prow = slot % P

    edst = np.full((NCORES, P, ncols), -1.0, np.float32)
    recp = np.zeros((NCORES, P, ncols), np.float32)
    cnt_dst = np.bincount(dst, minlength=NCORES * NPC).astype(np.float64)
    recip_dst = 1.0 / np.maximum(cnt_dst, 1.0)
    edst[co, prow, gcol] = (to * P + off[order]).astype(np.float32)
    recp[co, prow, gcol] = recip_dst[dst[order]].astype(np.float32)

    # int16 gather indices: slot i of a call at [i%16, i//16] within the
    # call's column range (x8 replicated over partition groups of 16)
    idx16 = np.zeros((NCORES, 16, ncols * 8), np.int16)
    callslot = gcol * P + prow - colbase[go, qo] * P
    callb = colbase[go, qo] * 8  # call col base *128/16 = *8
    idx16[co, callslot % 16, callb + callslot // 16] = qrow[order].astype(
        np.int16
    )
    idx16 = np.ascontiguousarray(np.tile(idx16, (1, 8, 1)))

    # incidence lists, core-independent: per (g, t): ordered (q, gcol)
    inc = [[[] for _ in range(TG)] for _ in range(NG)]
    for gg in range(NG):
        for t in range(TG):
            for qq in range(NQ):
                c = cap[gg, qq, t]
                if c == 0:
                    continue
                s0 = scum[gg, qq, t]
                s1 = s0 + c
                for cc in range(s0 // P, -(-s1 // P)):
                    inc[gg][t].append(int(colbase[gg, qq] + cc))
    return edst, recp, idx16, ncols, calls, inc


def _build(ncols, calls, inc):
    from concourse import bacc, bass, mybir, tile

    f16 = mybir.dt.float16
    f32 = mybir.dt.float32
    i16 = mybir.dt.int16
    EQ = mybir.AluOpType.is_equal
    MUL = mybir.AluOpType.mult

    nc = bacc.Bacc(
        "TRN2", target_bir_lowering=False, debug=False, num_devices=NCORES
    )

    xq = [
        nc.declare_dram_parameter(f"xq{i}", [QROWS, P], f16, isOutput=False)
        for i in range(NQ)
    ]
    xown = nc.declare_dram_parameter("xown", [NPC_PAD, P], f16, isOutput=False)
    idx_d = nc.declare_dram_parameter("idx16", [P, ncols * 8], i16, isOutput=False)
    edst_d = nc.declare_dram_parameter("edst", [P, ncols], f32, isOutput=False)
    recp_d = nc.declare_dram_parameter("recp", [P, ncols], f32, isOutput=False)
    iota_d = nc.declare_dram_parameter("iotat", [P, TG * P], f16, isOutput=False)
    wl1_d = nc.declare_dram_parameter("wl1", [P, P], f16, isOutput=False)
    wr1_d = nc.declare_dram_parameter("wr1", [P, P], f16, isOutput=False)
    wl2_d = nc.declare_dram_parameter("wl2", [P, P], f16, isOutput=False)
    wr2_d = nc.declare_dram_parameter("wr2", [P, P], f16, isOutput=False)
    b1_d = nc.declare_dram_parameter("b1", [1, P], f16, isOutput=False)
    b2_d = nc.declare_dram_parameter("b2", [1, P], f16, isOutput=False)
    ones_d = nc.declare_dram_parameter("ones1", [1, P], f16, isOutput=False)
    out_d = nc.declare_dram_parameter("out", [NPC_PAD, P], f32, isOutput=True)

    maxcallcols = max(c[3] for c in calls)

    with tile.TileContext(nc) as tc:
        with (
            tc.tile_pool(name="const", bufs=1) as cpool,
            tc.tile_pool(name="gath", bufs=2) as gpool,
            tc.tile_pool(name="xot", bufs=3) as tpool,
            tc.tile_pool(name="work", bufs=8) as wpool,
            tc.tile_pool(name="aggp", bufs=4) as apool,
            tc.tile_pool(name="outp", bufs=3) as opool,
            tc.tile_pool(name="psacc", bufs=4, space="PSUM") as ps_acc,
            tc.tile_pool(name="psh", bufs=3, space="PSUM") as ps_h,
            tc.tile_pool(name="dram", bufs=1, space="DRAM") as dpool,
        ):
            def cload(dram_ap, shape, dtype, name):
                t = cpool.tile(shape, dtype, name=name)
                nc.sync.dma_start(out=t[:], in_=dram_ap)
                return t

            wl1 = cload(wl1_d[:], [P, P], f16, "wl1")
            wr1 = cload(wr1_d[:], [P, P], f16, "wr1")
            wl2 = cload(wl2_d[:], [P, P], f16, "wl2")
            wr2 = cload(wr2_d[:], [P, P], f16, "wr2")
            b1 = cload(b1_d[:], [1, P], f16, "b1")
            b2 = cload(b2_d[:], [1, P], f16, "b2")
            ones1 = cload(ones_d[:], [1, P], f16, "ones1")
            iotat = cload(iota_d[:], [P, TG * P], f16, "iotat")
            edst = cload(edst_d[:], [P, ncols], f32, "edst")
            recp = cload(recp_d[:], [P, ncols], f32, "recp")
            idx_sb = cload(idx_d[:], [P, ncols * 8], i16, "idx16")

            h_bounce = dpool.tile([NPC_PAD, P], f16, name="h_bounce")
            h_full = dpool.tile([NALL, P], f16, name="h_full", addr_space="Shared")

            def layer(qtab, selftab, dst_dram, wl, wr, brow, relu):
                # per-block gather tiles keyed by q
                pend = []  # deferred h-phase closures (1-tile lookahead)

                def flush(n):
                    while len(pend) > n:
                        pend.pop(0)()

                for b in range(NB):
                    gs = [gg for gg in range(NG) if gg // GB == b]
                    btiles = {}
                    bbase = {}
                    for (bb, qq, c0, ccols) in calls:
                        if bb != b:
                            continue
                        gt = gpool.tile(
                            [P, maxcallcols, P], f16, tag=f"g{qq}"
                        )
                        nc.gpsimd.dma_gather(
                            gt[:, :ccols, :],
                            qtab[qq],
                            idx_sb[:, c0 * 8 : (c0 + ccols) * 8],
                            ccols * P,
                            ccols * P,
                            P,
                            single_packet=False,
                        )
                        btiles[qq] = gt
                        bbase[qq] = c0
                    for gg in gs:
                        xot = tpool.tile([P, TG * P], f16, tag="xot")
                        nc.sync.dma_start_transpose(
                            xot[:], selftab[gg * TG * P : (gg + 1) * TG * P, :]
                        )
                        for t in range(TG):
                            cols = inc[gg][t]
                            acc = None
                            if cols:
                                acc = ps_acc.tile([P, P], f32, tag="acc")
                                for ci, gc in enumerate(cols):
                                    # which call does this global col live in
                                    for (bb, qq, c0, ccols) in calls:
                                        if bb == b and c0 <= gc < c0 + ccols:
                                            break
                                    ind = wpool.tile([P, P], f16, tag="ind")
                                    nc.vector.tensor_scalar(
                                        out=ind[:],
                                        in0=iotat[:, t * P : (t + 1) * P],
                                        scalar1=edst[:, gc : gc + 1],
                                        scalar2=recp[:, gc : gc + 1],
                                        op0=EQ,
                                        op1=MUL,
                                    )
                                    nc.tensor.matmul(
                                        out=acc[:],
                                        lhsT=btiles[qq][:, gc - bbase[qq], :],
                                        rhs=ind[:],
                                        start=(ci == 0),
                                        stop=(ci == len(cols) - 1),
                                    )

                            def hphase(gg=gg, t=t, acc=acc, xot=xot):
                                tg_glob = gg * TG + t
                                hps = ps_h.tile([P, P], f32, tag="h")
                                nc.tensor.matmul(
                                    out=hps[:], lhsT=ones1[:], rhs=brow[:],
                                    start=True, stop=False,
                                )
                                if acc is not None:
                                    aggT = apool.tile([P, P], f16, tag="aggT")
                                    nc.scalar.activation(
                                        out=aggT[:], in_=acc[:],
                                        func=mybir.ActivationFunctionType.Copy,
                                    )
                                    nc.tensor.matmul(
                                        out=hps[:], lhsT=aggT[:], rhs=wl[:],
                                        start=False, stop=False,
                                    )
                                nc.tensor.matmul(
                                    out=hps[:],
                                    lhsT=xot[:, t * P : (t + 1) * P],
                                    rhs=wr[:],
                                    start=False, stop=True,
                                )
                                odt = f16 if relu else f32
                                hsb = opool.tile([P, P], odt, tag="hsb")
                                nc.scalar.activation(
                                    out=hsb[:], in_=hps[:],
                                    func=(
                                        mybir.ActivationFunctionType.Relu
                                        if relu
                                        else mybir.ActivationFunctionType.Copy
                                    ),
                                )
                                nc.sync.dma_start(
                                    out=dst_dram[
                                        tg_glob * P : (tg_glob + 1) * P, :
                                    ],
                                    in_=hsb[:],
                                )

                            pend.append(hphase)
                            flush(1)
                flush(0)

            xq_aps = [t[:, :] for t in xq]
            layer(xq_aps, xown, h_bounce, wl1, wr1, b1, relu=True)
            nc.gpsimd.collective_compute(
                "AllGather",
                mybir.AluOpType.bypass,
                replica_groups=[list(range(NCORES))],
                ins=[h_bounce[:]],
                outs=[h_full[:]],
            )
            hq_aps = [h_full[i * QROWS : (i + 1) * QROWS, :] for i in range(NQ)]
            layer(hq_aps, h_bounce, out_d, wl2, wr2, b2, relu=False)

    return nc


def run(x, edge_index, W_l1, b_l1, W_r1, W_l2, b_l2, W_r2, trace=False):
    n_nodes = x.shape[0]
    assert n_nodes == NCORES * NPC

    edst, recp, idx16, ncols, calls, inc = _prep(np.asarray(edge_index))

    x = np.asarray(x, np.float32)
    x_pad = np.zeros((NALL, P), np.float16)
    for c in range(NCORES):
        x_pad[c * NPC_PAD : c * NPC_PAD + NPC] = x[c * NPC : (c + 1) * NPC]

    iotat = np.tile(
        np.arange(TG * P, dtype=np.float16), (P, 1)
    )
    common = {
        **{f"xq{i}": np.ascontiguousarray(x_pad[i * QROWS : (i + 1) * QROWS])
           for i in range(NQ)},
        "wl1": np.asarray(W_l1, np.float16),
        "wr1": np.asarray(W_r1, np.float16),
        "wl2": np.asarray(W_l2, np.float16),
        "wr2": np.asarray(W_r2, np.float16),
        "b1": np.asarray(b_l1, np.float16).reshape(1, P),
        "b2": np.asarray(b_l2, np.float16).reshape(1, P),
        "ones1": np.ones((1, P), np.float16),
        "iotat": np.ascontiguousarray(iotat),
    }
    in_maps = []
    for c in range(NCORES):
        m = dict(common)
        m["xown"] = np.ascontiguousarray(x_pad[c * NPC_PAD : (c + 1) * NPC_PAD])
        m["idx16"] = idx16[c]
        m["edst"] = np.ascontiguousarray(edst[c])
        m["recp"] = np.ascontiguousarray(recp[c])
        in_maps.append(m)

    nc = _build(ncols, calls, inc)
    nc.finalize()

    from concourse.bass_utils import run_bass_kernel_spmd

    res = run_bass_kernel_spmd(nc, in_maps, list(range(NCORES)), trace=trace)
    out = np.empty((n_nodes, P), np.float32)
    for c in range(NCORES):
        out[c * NPC : (c + 1) * NPC] = res.results[c]["out"][:NPC]
    return out, res


def kernel(x, edge_index, W_l1, b_l1, W_r1, W_l2, b_l2, W_r2):
    out, _ = run(x, edge_index, W_l1, b_l1, W_r1, W_l2, b_l2, W_r2)
    return out


# revision 6
# speedup vs baseline: 1.6761x; 1.5203x over previous
"""Two-layer GraphSAGE (mean aggregation) on 8 Trainium2 NeuronCores.

Strategy (dst-partitioned, graph/data parallel):
- Nodes partitioned by destination across 8 cores (12500 each, padded to
  12544 = 98*128). x replicated per core in bf16 with a padded row space so
  src ids are layer-invariant.
- Edges bucketed per core by (group of 14 dst tiles, src quarter) and sorted
  by dst tile within each bucket. Source rows are fetched with large SWDGE
  dma_gather calls (int16 indices -> the padded node table is split into 4
  quarters of 25088 rows each); the 4 quarter calls of each block run on 4
  SWDGE queues so descriptor generation proceeds in parallel on the Q7s.
  Slot layout is identical on every core (capacity = max count over cores,
  pad slots gather row 0 with a zero indicator) so the SPMD program is
  uniform.
- Aggregation: per (column, dst tile) incidence, one DVE tensor_scalar
  builds a 0/1 indicator (iota == edst) and one PE matmul accumulates raw
  sums [feat, dst] into PSUM. Mean normalization is applied per dst tile
  (partition-scalar multiply by 1/indegree) after the W_l matmul.
- h tile = relu((sums^T @ W_l) * recip + bias + x_selfT @ W_r); self term
  loaded pre-transposed via HWDGE dma_start_transpose (bf16).
- AllGather (bf16) exchanges h between layers.

kernel(**inputs) -> np.ndarray takes FULL inputs, returns FULL [100000, 128]
float32 output; all sharding happens inside.
"""

import numpy as np

P = 128
NCORES = 8
NPC = 12500
TPC = 98
NPC_PAD = TPC * P            # 12544
NALL = NCORES * NPC_PAD      # 100352
NQ = 4
QROWS = NALL // NQ           # 25088
TG = 14                      # dst tiles per group
NG = TPC // TG               # 7 groups
GB = 2                       # groups per gather block
NB = -(-NG // GB)            # 4 gather blocks


def _prep(edge_index):
    """Host-side edge bucketing. Returns per-core SBUF tables and the
    core-independent program structure."""
    src = edge_index[0].astype(np.int64)
    dst = edge_index[1].astype(np.int64)
    core = dst // NPC
    loc = dst % NPC
    tl = loc // P
    off = loc % P
    g = tl // TG
    tl_loc = tl % TG
    srcpad = (src // NPC) * NPC_PAD + (src % NPC)
    q = srcpad // QROWS
    qrow = srcpad % QROWS

    key = ((core * NG + g) * NQ + q) * TG + tl_loc
    cnt = np.bincount(key, minlength=NCORES * NG * NQ * TG).reshape(
        NCORES, NG, NQ, TG
    )
    cap = cnt.max(axis=0)                       # [NG, NQ, TG]
    scum = np.zeros((NG, NQ, TG + 1), np.int64)
    np.cumsum(cap, axis=2, out=scum[:, :, 1:])
    segslots = scum[:, :, TG]
    segcols = -(-segslots // P)

    blocks = [list(range(b * GB, min((b + 1) * GB, NG))) for b in range(NB)]
    colbase = np.zeros((NG, NQ), np.int64)
    calls = []                                  # (b, q, col0, callcols)
    ncols = 0
    for b, gs in enumerate(blocks):
        for qq in range(NQ):
            c0 = ncols
            for gg in gs:
                colbase[gg, qq] = ncols
                ncols += int(segcols[gg, qq])
            calls.append((b, qq, c0, ncols - c0))

    order = np.lexsort((tl_loc, q, g, core))
    sk = key[order]
    first = np.r_[True, sk[1:] != sk[:-1]]
    idx_of_first = np.where(first)[0]
    grp_id = np.cumsum(first) - 1
    rank = np.arange(len(sk)) - idx_of_first[grp_id]
    go, qo, to, co = g[order], q[order], tl_loc[order], core[order]
    slot = scum[go, qo, to] + rank
    gcol = colbase[go, qo] + slot // P
    prow = slot % P

    edst = np.full((NCORES, P, ncols), -1.0, np.float32)
    cnt_dst = np.bincount(dst, minlength=NCORES * NPC).astype(np.float64)
    recip_dst = (1.0 / np.maximum(cnt_dst, 1.0)).astype(np.float32)
    edst[co, prow, gcol] = (to * P + off[order]).astype(np.float32)
    # per-dst-node reciprocal in tile layout: [core, P(part), TPC(tile)]
    recip_t = np.zeros((NCORES, P, TPC), np.float32)
    for c in range(NCORES):
        r = np.zeros(NPC_PAD, np.float32)
        r[:NPC] = recip_dst[c * NPC : (c + 1) * NPC]
        recip_t[c] = r.reshape(TPC, P).T

    idx16 = np.zeros((NCORES, 16, ncols * 8), np.int16)
    callslot = gcol * P + prow - colbase[go, qo] * P
    callb = colbase[go, qo] * 8
    idx16[co, callslot % 16, callb + callslot // 16] = qrow[order].astype(
        np.int16
    )
    idx16 = np.ascontiguousarray(np.tile(idx16, (1, 8, 1)))

    inc = [[[] for _ in range(TG)] for _ in range(NG)]
    for gg in range(NG):
        for t in range(TG):
            for qq in range(NQ):
                c = cap[gg, qq, t]
                if c == 0:
                    continue
                s0 = scum[gg, qq, t]
                s1 = s0 + c
                for cc in range(s0 // P, -(-s1 // P)):
                    inc[gg][t].append(int(colbase[gg, qq] + cc))
    return edst, recip_t, idx16, ncols, calls, inc


def _build(ncols, calls, inc):
    from concourse import bacc, bass, mybir, tile

    bf16 = mybir.dt.bfloat16
    f16 = mybir.dt.float16
    f32 = mybir.dt.float32
    i16 = mybir.dt.int16
    EQ = mybir.AluOpType.is_equal

    nc = bacc.Bacc(
        "TRN2", target_bir_lowering=False, debug=False, num_devices=NCORES,
        num_swdge_queues=4,
    )

    xq = [
        nc.declare_dram_parameter(f"xq{i}", [QROWS, P], bf16, isOutput=False)
        for i in range(NQ)
    ]
    xown = nc.declare_dram_parameter("xown", [NPC_PAD, P], bf16, isOutput=False)
    idx_d = nc.declare_dram_parameter("idx16", [P, ncols * 8], i16, isOutput=False)
    edst_d = nc.declare_dram_parameter("edst", [P, ncols], f32, isOutput=False)
    recp_d = nc.declare_dram_parameter("recp", [P, TPC], f32, isOutput=False)
    iota_d = nc.declare_dram_parameter("iotat", [P, TG * P], f16, isOutput=False)
    wl1_d = nc.declare_dram_parameter("wl1", [P, P], bf16, isOutput=False)
    wr1_d = nc.declare_dram_parameter("wr1", [P, P], bf16, isOutput=False)
    wl2_d = nc.declare_dram_parameter("wl2", [P, P], bf16, isOutput=False)
    wr2_d = nc.declare_dram_parameter("wr2", [P, P], bf16, isOutput=False)
    b1_d = nc.declare_dram_parameter("b1", [1, P], bf16, isOutput=False)
    b2_d = nc.declare_dram_parameter("b2", [1, P], bf16, isOutput=False)
    ones_d = nc.declare_dram_parameter("ones1", [1, P], bf16, isOutput=False)
    out_d = nc.declare_dram_parameter("out", [NPC_PAD, P], f32, isOutput=True)

    maxcallcols = max(c[3] for c in calls)

    with tile.TileContext(nc) as tc:
        with (
            tc.tile_pool(name="const", bufs=1) as cpool,
            tc.tile_pool(name="gath", bufs=2) as gpool,
            tc.tile_pool(name="xot", bufs=3) as tpool,
            tc.tile_pool(name="work", bufs=8) as wpool,
            tc.tile_pool(name="aggp", bufs=4) as apool,
            tc.tile_pool(name="scl", bufs=3) as spool,
            tc.tile_pool(name="outp", bufs=3) as opool,
            tc.tile_pool(name="psacc", bufs=3, space="PSUM") as ps_acc,
            tc.tile_pool(name="psh1", bufs=2, space="PSUM") as ps_h1,
            tc.tile_pool(name="psh2", bufs=2, space="PSUM") as ps_h2,
            tc.tile_pool(name="dram", bufs=1, space="DRAM") as dpool,
        ):
            def cload(dram_ap, shape, dtype, name):
                t = cpool.tile(shape, dtype, name=name)
                nc.sync.dma_start(out=t[:], in_=dram_ap)
                return t

            wl1 = cload(wl1_d[:], [P, P], bf16, "wl1")
            wr1 = cload(wr1_d[:], [P, P], bf16, "wr1")
            wl2 = cload(wl2_d[:], [P, P], bf16, "wl2")
            wr2 = cload(wr2_d[:], [P, P], bf16, "wr2")
            b1 = cload(b1_d[:], [1, P], bf16, "b1")
            b2 = cload(b2_d[:], [1, P], bf16, "b2")
            ones1 = cload(ones_d[:], [1, P], bf16, "ones1")
            iotat = cload(iota_d[:], [P, TG * P], f16, "iotat")
            edst = cload(edst_d[:], [P, ncols], f32, "edst")
            recp = cload(recp_d[:], [P, TPC], f32, "recp")
            idx_sb = cload(idx_d[:], [P, ncols * 8], i16, "idx16")

            h_bounce = dpool.tile([NPC_PAD, P], bf16, name="h_bounce")
            h_full = dpool.tile([NALL, P], bf16, name="h_full", addr_space="Shared")

            def layer(qtab, selftab, dst_dram, wl, wr, brow, relu):
                pend = []

                def flush(n):
                    while len(pend) > n:
                        pend.pop(0)()

                for b in range(NB):
                    gs = [gg for gg in range(NG) if gg // GB == b]
                    btiles = {}
                    bbase = {}
                    for (bb, qq, c0, ccols) in calls:
                        if bb != b:
                            continue
                        gt = gpool.tile([P, maxcallcols, P], bf16, tag=f"g{qq}")
                        nc.gpsimd.dma_gather(
                            gt[:, :ccols, :],
                            qtab[qq],
                            idx_sb[:, c0 * 8 : (c0 + ccols) * 8],
                            ccols * P,
                            ccols * P,
                            P,
                            single_packet=False,
                            queue_num=qq,
                        )
                        btiles[qq] = gt
                        bbase[qq] = c0
                    for gg in gs:
                        xot = tpool.tile([P, TG * P], bf16, tag="xot")
                        nc.sync.dma_start_transpose(
                            xot[:], selftab[gg * TG * P : (gg + 1) * TG * P, :]
                        )
                        for t in range(TG):
                            cols = inc[gg][t]
                            acc = None
                            if cols:
                                acc = ps_acc.tile([P, P], f32, tag="acc")
                                for ci, gc in enumerate(cols):
                                    for (bb, qq, c0, ccols) in calls:
                                        if bb == b and c0 <= gc < c0 + ccols:
                                            break
                                    ind = wpool.tile([P, P], bf16, tag="ind")
                                    nc.vector.tensor_scalar(
                                        out=ind[:],
                                        in0=iotat[:, t * P : (t + 1) * P],
                                        scalar1=edst[:, gc : gc + 1],
                                        scalar2=None,
                                        op0=EQ,
                                    )
                                    nc.tensor.matmul(
                                        out=acc[:],
                                        lhsT=btiles[qq][:, gc - bbase[qq], :],
                                        rhs=ind[:],
                                        start=(ci == 0),
                                        stop=(ci == len(cols) - 1),
                                    )

                            def hphase(gg=gg, t=t, acc=acc, xot=xot):
                                tg_glob = gg * TG + t
                                # h2 = bias + selfT @ Wr
                                h2 = ps_h2.tile([P, P], f32, tag="h2")
                                nc.tensor.matmul(
                                    out=h2[:], lhsT=ones1[:], rhs=brow[:],
                                    start=True, stop=False,
                                )
                                nc.tensor.matmul(
                                    out=h2[:],
                                    lhsT=xot[:, t * P : (t + 1) * P],
                                    rhs=wr[:],
                                    start=False, stop=True,
                                )
                                if acc is not None:
                                    aggT = apool.tile([P, P], bf16, tag="aggT")
                                    nc.scalar.activation(
                                        out=aggT[:], in_=acc[:],
                                        func=mybir.ActivationFunctionType.Copy,
                                    )
                                    h1 = ps_h1.tile([P, P], f32, tag="h1")
                                    nc.tensor.matmul(
                                        out=h1[:], lhsT=aggT[:], rhs=wl[:],
                                        start=True, stop=True,
                                    )
                                    t1 = spool.tile([P, P], f32, tag="t1")
                                    nc.vector.tensor_scalar_mul(
                                        out=t1[:], in0=h1[:],
                                        scalar1=recp[:, tg_glob : tg_glob + 1],
                                    )
                                    tsum = spool.tile([P, P], f32, tag="tsum")
                                    nc.vector.tensor_add(
                                        out=tsum[:], in0=t1[:], in1=h2[:]
                                    )
                                else:
                                    tsum = spool.tile([P, P], f32, tag="tsum")
                                    nc.vector.tensor_copy(
                                        out=tsum[:], in_=h2[:]
                                    )
                                odt = bf16 if relu else f32
                                hsb = opool.tile([P, P], odt, tag="hsb")
                                nc.scalar.activation(
                                    out=hsb[:], in_=tsum[:],
                                    func=(
                                        mybir.ActivationFunctionType.Relu
                                        if relu
                                        else mybir.ActivationFunctionType.Copy
                                    ),
                                )
                                nc.sync.dma_start(
                                    out=dst_dram[
                                        tg_glob * P : (tg_glob + 1) * P, :
                                    ],
                                    in_=hsb[:],
                                )

                            pend.append(hphase)
                            flush(1)
                flush(0)

            xq_aps = [t[:, :] for t in xq]
            layer(xq_aps, xown, h_bounce, wl1, wr1, b1, relu=True)
            nc.gpsimd.collective_compute(
                "AllGather",
                mybir.AluOpType.bypass,
                replica_groups=[list(range(NCORES))],
                ins=[h_bounce[:]],
                outs=[h_full[:]],
            )
            hq_aps = [h_full[i * QROWS : (i + 1) * QROWS, :] for i in range(NQ)]
            layer(hq_aps, h_bounce, out_d, wl2, wr2, b2, relu=False)

    return nc


def run(x, edge_index, W_l1, b_l1, W_r1, W_l2, b_l2, W_r2, trace=False):
    import ml_dtypes

    bf = ml_dtypes.bfloat16
    n_nodes = x.shape[0]
    assert n_nodes == NCORES * NPC

    edst, recip_t, idx16, ncols, calls, inc = _prep(np.asarray(edge_index))

    x = np.asarray(x, np.float32)
    x_pad = np.zeros((NALL, P), bf)
    for c in range(NCORES):
        x_pad[c * NPC_PAD : c * NPC_PAD + NPC] = x[c * NPC : (c + 1) * NPC]

    iotat = np.tile(np.arange(TG * P, dtype=np.float16), (P, 1))
    common = {
        **{f"xq{i}": np.ascontiguousarray(x_pad[i * QROWS : (i + 1) * QROWS])
           for i in range(NQ)},
        "wl1": np.asarray(W_l1, bf),
        "wr1": np.asarray(W_r1, bf),
        "wl2": np.asarray(W_l2, bf),
        "wr2": np.asarray(W_r2, bf),
        "b1": np.asarray(b_l1, bf).reshape(1, P),
        "b2": np.asarray(b_l2, bf).reshape(1, P),
        "ones1": np.ones((1, P), bf),
        "iotat": np.ascontiguousarray(iotat),
    }
    in_maps = []
    for c in range(NCORES):
        m = dict(common)
        m["xown"] = np.ascontiguousarray(x_pad[c * NPC_PAD : (c + 1) * NPC_PAD])
        m["idx16"] = idx16[c]
        m["edst"] = np.ascontiguousarray(edst[c])
        m["recp"] = np.ascontiguousarray(recip_t[c])
        in_maps.append(m)

    nc = _build(ncols, calls, inc)
    nc.finalize()

    from concourse.bass_utils import run_bass_kernel_spmd

    res = run_bass_kernel_spmd(nc, in_maps, list(range(NCORES)), trace=trace)
    out = np.empty((n_nodes, P), np.float32)
    for c in range(NCORES):
        out[c * NPC : (c + 1) * NPC] = res.results[c]["out"][:NPC]
    return out, res


def kernel(x, edge_index, W_l1, b_l1, W_r1, W_l2, b_l2, W_r2):
    out, _ = run(x, edge_index, W_l1, b_l1, W_r1, W_l2, b_l2, W_r2)
    return out


# revision 8
# speedup vs baseline: 1.7867x; 1.0660x over previous
"""Two-layer GraphSAGE (mean aggregation) on 8 Trainium2 NeuronCores.

Strategy (dst-partitioned, graph/data parallel):
- Nodes partitioned by destination across 8 cores (12500 each, padded to
  12544 = 98*128). x replicated per core in bf16. The padded global row
  space is [half, core, 6272] so the between-layer AllGather can be split
  into two half-collectives whose outputs are exactly quarters 0-1 / 2-3 of
  the gather table (quarter = 25088 rows < int16 range).
- Edges bucketed per core by (group of 14 dst tiles, src quarter), sorted by
  dst tile. Source rows are fetched with large SWDGE dma_gather calls; the
  4 quarter calls of each block run on 4 SWDGE queues so Q7 descriptor
  generation proceeds in parallel. Slot layout is identical on every core
  (capacity = max count over cores; pad slots gather row 0 with a zero
  indicator) keeping the SPMD program uniform.
- Aggregation: per (column, dst tile) incidence, one DVE tensor_scalar
  builds a 0/1 indicator (iota == edst) and one PE matmul accumulates raw
  sums [feat, dst] into PSUM. Mean normalization is a per-partition scalar
  multiply after the W_l matmul.
- h tile = relu((sums^T @ W_l) * recip + bias + x_selfT @ W_r); self term
  loaded pre-transposed via HWDGE dma_start_transpose (bf16). The layer-2
  gathers for quarters 0-1 only wait on the first half-AllGather.

kernel(**inputs) -> np.ndarray takes FULL inputs, returns FULL [100000, 128]
float32 output; all sharding happens inside.
"""

import numpy as np

P = 128
NCORES = 8
NPC = 12500
TPC = 98
NPC_PAD = TPC * P            # 12544
NALL = NCORES * NPC_PAD      # 100352
NQ = 4
QROWS = NALL // NQ           # 25088
HROWS = NPC_PAD // 2         # 6272 rows per half per core (= 49 tiles)
TG = 14                      # dst tiles per group
NG = TPC // TG               # 7 groups
GB = 2                       # groups per gather block
NB = -(-NG // GB)            # 4 gather blocks


def _prep(edge_index):
    src = edge_index[0].astype(np.int64)
    dst = edge_index[1].astype(np.int64)
    core = dst // NPC
    loc = dst % NPC
    tl = loc // P
    off = loc % P
    g = tl // TG
    tl_loc = tl % TG
    # padded global row space: [half, core, HROWS]
    sloc = src % NPC
    half = sloc // HROWS
    srcpad = half * (NCORES * HROWS) + (src // NPC) * HROWS + sloc % HROWS
    q = srcpad // QROWS
    qrow = srcpad % QROWS

    key = ((core * NG + g) * NQ + q) * TG + tl_loc
    cnt = np.bincount(key, minlength=NCORES * NG * NQ * TG).reshape(
        NCORES, NG, NQ, TG
    )
    cap = cnt.max(axis=0)
    scum = np.zeros((NG, NQ, TG + 1), np.int64)
    np.cumsum(cap, axis=2, out=scum[:, :, 1:])
    segcols = -(-scum[:, :, TG] // P)

    blocks = [list(range(b * GB, min((b + 1) * GB, NG))) for b in range(NB)]
    colbase = np.zeros((NG, NQ), np.int64)
    calls = []                                  # (b, q, col0, callcols)
    ncols = 0
    for b, gs in enumerate(blocks):
        for qq in range(NQ):
            c0 = ncols
            for gg in gs:
                colbase[gg, qq] = ncols
                ncols += int(segcols[gg, qq])
            calls.append((b, qq, c0, ncols - c0))

    order = np.lexsort((tl_loc, q, g, core))
    sk = key[order]
    first = np.r_[True, sk[1:] != sk[:-1]]
    idx_of_first = np.where(first)[0]
    grp_id = np.cumsum(first) - 1
    rank = np.arange(len(sk)) - idx_of_first[grp_id]
    go, qo, to, co = g[order], q[order], tl_loc[order], core[order]
    slot = scum[go, qo, to] + rank
    gcol = colbase[go, qo] + slot // P
    prow = slot % P

    edst = np.full((NCORES, P, ncols), -1.0, np.float32)
    cnt_dst = np.bincount(dst, minlength=NCORES * NPC).astype(np.float64)
    recip_dst = (1.0 / np.maximum(cnt_dst, 1.0)).astype(np.float32)
    edst[co, prow, gcol] = (to * P + off[order]).astype(np.float32)
    recip_t = np.zeros((NCORES, P, TPC), np.float32)
    for c in range(NCORES):
        r = np.zeros(NPC_PAD, np.float32)
        r[:NPC] = recip_dst[c * NPC : (c + 1) * NPC]
        recip_t[c] = r.reshape(TPC, P).T

    idx16 = np.zeros((NCORES, 16, ncols * 8), np.int16)
    callslot = gcol * P + prow - colbase[go, qo] * P
    callb = colbase[go, qo] * 8
    idx16[co, callslot % 16, callb + callslot // 16] = qrow[order].astype(
        np.int16
    )
    idx16 = np.ascontiguousarray(np.tile(idx16, (1, 8, 1)))

    inc = [[[] for _ in range(TG)] for _ in range(NG)]
    for gg in range(NG):
        for t in range(TG):
            for qq in range(NQ):
                c = cap[gg, qq, t]
                if c == 0:
                    continue
                s0 = scum[gg, qq, t]
                for cc in range(s0 // P, -(-(s0 + c) // P)):
                    inc[gg][t].append(int(colbase[gg, qq] + cc))
    return edst, recip_t, idx16, ncols, calls, inc


def _build(ncols, calls, inc, skip_bias):
    from concourse import bacc, bass, mybir, tile

    bf16 = mybir.dt.bfloat16
    f16 = mybir.dt.float16
    f32 = mybir.dt.float32
    i16 = mybir.dt.int16
    EQ = mybir.AluOpType.is_equal

    nc = bacc.Bacc(
        "TRN2", target_bir_lowering=False, debug=False, num_devices=NCORES,
        num_swdge_queues=4,
    )

    # x table: one tensor per half, each [2*QROWS, P]; quarter views inside
    xh = [
        nc.declare_dram_parameter(f"xh{i}", [2 * QROWS, P], bf16, isOutput=False)
        for i in range(2)
    ]
    xown = nc.declare_dram_parameter("xown", [NPC_PAD, P], bf16, isOutput=False)
    idx_d = nc.declare_dram_parameter("idx16", [P, ncols * 8], i16, isOutput=False)
    edst_d = nc.declare_dram_parameter("edst", [P, ncols], f32, isOutput=False)
    recp_d = nc.declare_dram_parameter("recp", [P, TPC], f32, isOutput=False)
    iota_d = nc.declare_dram_parameter("iotat", [P, TG * P], f16, isOutput=False)
    wl1_d = nc.declare_dram_parameter("wl1", [P, P], bf16, isOutput=False)
    wr1_d = nc.declare_dram_parameter("wr1", [P, P], bf16, isOutput=False)
    wl2_d = nc.declare_dram_parameter("wl2", [P, P], bf16, isOutput=False)
    wr2_d = nc.declare_dram_parameter("wr2", [P, P], bf16, isOutput=False)
    b1_d = nc.declare_dram_parameter("b1", [1, P], bf16, isOutput=False)
    b2_d = nc.declare_dram_parameter("b2", [1, P], bf16, isOutput=False)
    ones_d = nc.declare_dram_parameter("ones1", [1, P], bf16, isOutput=False)
    out_d = nc.declare_dram_parameter("out", [NPC_PAD, P], f32, isOutput=True)

    maxcallcols = max(c[3] for c in calls)

    with tile.TileContext(nc) as tc:
        with (
            tc.tile_pool(name="const", bufs=1) as cpool,
            tc.tile_pool(name="gath", bufs=2) as gpool,
            tc.tile_pool(name="xot", bufs=3) as tpool,
            tc.tile_pool(name="work", bufs=16) as wpool,
            tc.tile_pool(name="aggp", bufs=6) as apool,
            tc.tile_pool(name="scl", bufs=6) as spool,
            tc.tile_pool(name="outp", bufs=4) as opool,
            tc.tile_pool(name="psacc", bufs=3, space="PSUM") as ps_acc,
            tc.tile_pool(name="psh1", bufs=2, space="PSUM") as ps_h1,
            tc.tile_pool(name="psh2", bufs=2, space="PSUM") as ps_h2,
            tc.tile_pool(name="dram", bufs=1, space="DRAM") as dpool,
        ):
            def cload(dram_ap, shape, dtype, name):
                t = cpool.tile(shape, dtype, name=name)
                nc.sync.dma_start(out=t[:], in_=dram_ap)
                return t

            wl1 = cload(wl1_d[:], [P, P], bf16, "wl1")
            wr1 = cload(wr1_d[:], [P, P], bf16, "wr1")
            wl2 = cload(wl2_d[:], [P, P], bf16, "wl2")
            wr2 = cload(wr2_d[:], [P, P], bf16, "wr2")
            b1 = cload(b1_d[:], [1, P], bf16, "b1")
            b2 = cload(b2_d[:], [1, P], bf16, "b2")
            ones1 = cload(ones_d[:], [1, P], bf16, "ones1")
            iotat = cload(iota_d[:], [P, TG * P], f16, "iotat")
            edst = cload(edst_d[:], [P, ncols], f32, "edst")
            recp = cload(recp_d[:], [P, TPC], f32, "recp")
            idx_sb = cload(idx_d[:], [P, ncols * 8], i16, "idx16")

            h_bounce = dpool.tile([NPC_PAD, P], bf16, name="h_bounce")
            h_half = [
                dpool.tile([2 * QROWS, P], bf16, name=f"h_half{i}",
                           addr_space="Shared")
                for i in range(2)
            ]

            def layer(qtab, selftab, dst_dram, wl, wr, brow, relu):
                pend = []

                def flush(n):
                    while len(pend) > n:
                        pend.pop(0)()

                for b in range(NB):
                    gs = [gg for gg in range(NG) if gg // GB == b]
                    btiles = {}
                    bbase = {}
                    for (bb, qq, c0, ccols) in calls:
                        if bb != b:
                            continue
                        gt = gpool.tile([P, maxcallcols, P], bf16, tag=f"g{qq}")
                        nc.gpsimd.dma_gather(
                            gt[:, :ccols, :],
                            qtab[qq],
                            idx_sb[:, c0 * 8 : (c0 + ccols) * 8],
                            ccols * P,
                            ccols * P,
                            P,
                            single_packet=False,
                            queue_num=qq,
                        )
                        btiles[qq] = gt
                        bbase[qq] = c0
                    for gg in gs:
                        xot = tpool.tile([P, TG * P], bf16, tag="xot")
                        nc.sync.dma_start_transpose(
                            xot[:], selftab[gg * TG * P : (gg + 1) * TG * P, :]
                        )
                        for t in range(TG):
                            cols = inc[gg][t]
                            acc = None
                            if cols:
                                acc = ps_acc.tile([P, P], f32, tag="acc")
                                for ci, gc in enumerate(cols):
                                    for (bb, qq, c0, ccols) in calls:
                                        if bb == b and c0 <= gc < c0 + ccols:
                                            break
                                    ind = wpool.tile([P, P], bf16, tag="ind")
                                    nc.vector.tensor_scalar(
                                        out=ind[:],
                                        in0=iotat[:, t * P : (t + 1) * P],
                                        scalar1=edst[:, gc : gc + 1],
                                        scalar2=None,
                                        op0=EQ,
                                    )
                                    nc.tensor.matmul(
                                        out=acc[:],
                                        lhsT=btiles[qq][:, gc - bbase[qq], :],
                                        rhs=ind[:],
                                        start=(ci == 0),
                                        stop=(ci == len(cols) - 1),
                                    )

                            def hphase(gg=gg, t=t, acc=acc, xot=xot):
                                tg_glob = gg * TG + t
                                h2 = ps_h2.tile([P, P], f32, tag="h2")
                                if not skip_bias:
                                    nc.tensor.matmul(
                                        out=h2[:], lhsT=ones1[:], rhs=brow[:],
                                        start=True, stop=False,
                                    )
                                nc.tensor.matmul(
                                    out=h2[:],
                                    lhsT=xot[:, t * P : (t + 1) * P],
                                    rhs=wr[:],
                                    start=skip_bias, stop=True,
                                )
                                if acc is not None:
                                    aggT = apool.tile([P, P], bf16, tag="aggT")
                                    nc.scalar.activation(
                                        out=aggT[:], in_=acc[:],
                                        func=mybir.ActivationFunctionType.Copy,
                                    )
                                    h1 = ps_h1.tile([P, P], f32, tag="h1")
                                    nc.tensor.matmul(
                                        out=h1[:], lhsT=aggT[:], rhs=wl[:],
                                        start=True, stop=True,
                                    )
                                    # mean normalization: per-partition (dst)
                                    # scale on the Scalar engine
                                    t1 = spool.tile([P, P], f32, tag="t1")
                                    nc.scalar.activation(
                                        out=t1[:], in_=h1[:],
                                        func=mybir.ActivationFunctionType.Copy,
                                        scale=recp[:, tg_glob : tg_glob + 1],
                                    )
                                    if relu:
                                        tsum = spool.tile([P, P], f32, tag="ts")
                                        nc.vector.tensor_add(
                                            out=tsum[:], in0=t1[:], in1=h2[:]
                                        )
                                        hsb = opool.tile([P, P], bf16, tag="hs")
                                        nc.scalar.activation(
                                            out=hsb[:], in_=tsum[:],
                                            func=mybir.ActivationFunctionType.Relu,
                                        )
                                    else:
                                        hsb = opool.tile([P, P], f32, tag="hs32")
                                        nc.vector.tensor_add(
                                            out=hsb[:], in0=t1[:], in1=h2[:]
                                        )
                                else:
                                    if relu:
                                        hsb = opool.tile([P, P], bf16, tag="hs")
                                        nc.scalar.activation(
                                            out=hsb[:], in_=h2[:],
                                            func=mybir.ActivationFunctionType.Relu,
                                        )
                                    else:
                                        hsb = opool.tile([P, P], f32, tag="hs32")
                                        nc.vector.tensor_copy(
                                            out=hsb[:], in_=h2[:]
                                        )
                                nc.sync.dma_start(
                                    out=dst_dram[
                                        tg_glob * P : (tg_glob + 1) * P, :
                                    ],
                                    in_=hsb[:],
                                )

                            pend.append(hphase)
                            flush(2)
                flush(0)

            xq_aps = [
                xh[0][0:QROWS, :], xh[0][QROWS : 2 * QROWS, :],
                xh[1][0:QROWS, :], xh[1][QROWS : 2 * QROWS, :],
            ]
            layer(xq_aps, xown, h_bounce, wl1, wr1, b1, relu=True)
            for i in range(2):
                nc.gpsimd.collective_compute(
                    "AllGather",
                    mybir.AluOpType.bypass,
                    replica_groups=[list(range(NCORES))],
                    ins=[h_bounce[i * HROWS : (i + 1) * HROWS, :]],
                    outs=[h_half[i][:]],
                )
            hq_aps = [
                h_half[0][0:QROWS, :], h_half[0][QROWS : 2 * QROWS, :],
                h_half[1][0:QROWS, :], h_half[1][QROWS : 2 * QROWS, :],
            ]
            layer(hq_aps, h_bounce, out_d, wl2, wr2, b2, relu=False)

    return nc


def run(x, edge_index, W_l1, b_l1, W_r1, W_l2, b_l2, W_r2, trace=False):
    import ml_dtypes

    bf = ml_dtypes.bfloat16
    n_nodes = x.shape[0]
    assert n_nodes == NCORES * NPC

    edst, recip_t, idx16, ncols, calls, inc = _prep(np.asarray(edge_index))

    x = np.asarray(x, np.float32)
    # per-core padded slices, then relayout to [half, core, HROWS]
    xp = np.zeros((NCORES, NPC_PAD, P), bf)
    for c in range(NCORES):
        xp[c, :NPC] = x[c * NPC : (c + 1) * NPC]
    x_pad = np.ascontiguousarray(
        xp.reshape(NCORES, 2, HROWS, P).transpose(1, 0, 2, 3)
    ).reshape(NALL, P)

    iotat = np.tile(np.arange(TG * P, dtype=np.float16), (P, 1))
    skip_bias = not (np.any(np.asarray(b_l1)) or np.any(np.asarray(b_l2)))
    common = {
        "xh0": np.ascontiguousarray(x_pad[: NALL // 2]),
        "xh1": np.ascontiguousarray(x_pad[NALL // 2 :]),
        "wl1": np.asarray(W_l1, bf),
        "wr1": np.asarray(W_r1, bf),
        "wl2": np.asarray(W_l2, bf),
        "wr2": np.asarray(W_r2, bf),
        "b1": np.asarray(b_l1, bf).reshape(1, P),
        "b2": np.asarray(b_l2, bf).reshape(1, P),
        "ones1": np.ones((1, P), bf),
        "iotat": np.ascontiguousarray(iotat),
    }
    in_maps = []
    for c in range(NCORES):
        m = dict(common)
        m["xown"] = np.ascontiguousarray(xp[c])
        m["idx16"] = idx16[c]
        m["edst"] = np.ascontiguousarray(edst[c])
        m["recp"] = np.ascontiguousarray(recip_t[c])
        in_maps.append(m)

    nc = _build(ncols, calls, inc, skip_bias)
    nc.finalize()

    from concourse.bass_utils import run_bass_kernel_spmd

    res = run_bass_kernel_spmd(nc, in_maps, list(range(NCORES)), trace=trace)
    out = np.empty((n_nodes, P), np.float32)
    for c in range(NCORES):
        out[c * NPC : (c + 1) * NPC] = res.results[c]["out"][:NPC]
    return out, res


def kernel(x, edge_index, W_l1, b_l1, W_r1, W_l2, b_l2, W_r2):
    out, _ = run(x, edge_index, W_l1, b_l1, W_r1, W_l2, b_l2, W_r2)
    return out
